# revision 33
# baseline (speedup 1.0000x reference)
"""Trainium2 Bass kernel for LongNet-style dilated attention (B=2, T=4096, E=1024, H=16).

Sharding: 8 cores = 2 batches x 4 head-groups. Core (b, m) handles batch b and
heads {m, 4+m, 8+m, 12+m}. One SPMD program: the attention stage is emitted 4x
under tc.If(m == c) branches so every dilation-group offset is compile-time
static (register-offset APs only work at partition base 0 on TRN2, so dynamic
slicing is not usable for the odd head slots that live at partitions 64-127).

Math: for every scale, segment sparse length s = w/r = 256, so each (head,
scale, segment) is one [256 x 256 x 64] causal attention block. The reference's
lse-softmax merge over scales reduces algebraically to
    y[t,h] = (sum_i rawout_i[t,h]) / (sum_i Z_i[t,h])
over covering scales, with rawout = exp(s)@v and Z = rowsum(exp(s)),
accumulated directly in PSUM by aligning all scales' segment grids on 256-wide
t-blocks.  No exp/log at merge time; no per-scale normalization; Z comes from a
col-tiled all-ones matmul replicated across 64 partitions so the final
normalize is a partition-aligned reciprocal+multiply.
"""

import numpy as np
import ml_dtypes

B, T, E, H, D = 2, 4096, 1024, 16, 64
WS = (256, 512, 1024, 2048, 4096)
DILS = (1, 2, 4, 8, 16)
NBLK = 16
NCORES = 8

_RUNTIME = None


# ---------------------------------------------------------------- program ----
def _emit(nc, tc, ctx, mcase, debug=False, scales_on=(0,1,2,3,4), parts=('s1','s2','norm','s3'), ebufs=4, xbufs=3, obufs=4):
    import concourse.mybir as mybir
    from concourse.bass import _add_dep_helper

    def chain(insts):
        # enforce PE program order for matmuls sharing a PSUM accumulation bank
        for a, b_ in zip(insts[1:], insts[:-1]):
            _add_dep_helper(a.ins, b_.ins, False, "psum group order")

    bf16 = mybir.dt.bfloat16
    f32 = mybir.dt.float32
    Exp = mybir.ActivationFunctionType.Exp

    xT = nc.dram_tensor("xT", [E, T], bf16, kind="ExternalInput")
    wq_d = nc.dram_tensor("wq", [E, 256], bf16, kind="ExternalInput")
    wk_d = nc.dram_tensor("wk", [E, 256], bf16, kind="ExternalInput")
    wv_d = nc.dram_tensor("wv", [E, 256], bf16, kind="ExternalInput")
    wo_d = nc.dram_tensor("wo", [256, E], bf16, kind="ExternalInput")
    id_d = nc.dram_tensor("ident", [128, 128], bf16, kind="ExternalInput")
    tri_d = nc.dram_tensor("trimask", [128, 128], bf16, kind="ExternalInput")
    out_d = nc.dram_tensor("out", [T, E], f32, kind="ExternalOutput")

    consts = ctx.enter_context(tc.tile_pool(name="consts", bufs=1))
    wq_sb = consts.tile([128, 8, 256], bf16)
    wk_sb = consts.tile([128, 8, 256], bf16)
    wv_sb = consts.tile([128, 8, 256], bf16)
    wo_sb = consts.tile([128, 2, 1024], bf16)
    ident = consts.tile([128, 128], bf16)
    trimask = consts.tile([128, 128], bf16)
    ones64 = consts.tile([128, 64], bf16)
    nc.sync.dma_start(out=wq_sb, in_=wq_d.rearrange("(k p) m -> p k m", p=128))
    nc.sync.dma_start(out=wk_sb, in_=wk_d.rearrange("(k p) m -> p k m", p=128))
    nc.sync.dma_start(out=wv_sb, in_=wv_d.rearrange("(k p) m -> p k m", p=128))
    nc.sync.dma_start(out=wo_sb, in_=wo_d.rearrange("(k p) m -> p k m", p=128))
    nc.sync.dma_start(out=ident, in_=id_d[:])
    nc.sync.dma_start(out=trimask, in_=tri_d[:])
    nc.vector.memset(ones64, 1.0)

    big = ctx.enter_context(tc.tile_pool(name="big", bufs=1))
    qT = [big.tile([128, T], bf16, tag=f"qT{p}", name=f"qT{p}") for p in (0, 1)]
    kT = [big.tile([128, T], bf16, tag=f"kT{p}", name=f"kT{p}") for p in (0, 1)]
    vT = [big.tile([128, T], bf16, tag=f"vT{p}", name=f"vT{p}") for p in (0, 1)]
    yT = [big.tile([128, T], bf16, tag=f"yT{p}", name=f"yT{p}") for p in (0, 1)]

    # ------------------------------------------------ stage 1: projections ---
    if 's1' not in parts:
        for tl in (*qT, *kT, *vT):
            nc.sync.dma_start(out=tl, in_=xT[0:128, :])
    with tc.tile_pool(name="s1psum", bufs=2, space="PSUM") as s1psum, \
         tc.tile_pool(name="xin", bufs=2) as xin:
        xTr = xT.rearrange("(k p) t -> p k t", p=128)
        for t5 in (range(8) if 's1' in parts else []):
            xt = xin.tile([128, 8, 512], bf16, tag="xt", bufs=xbufs)
            nc.sync.dma_start(out=xt, in_=xTr[:, :, t5 * 512 : (t5 + 1) * 512])
            ev = 0
            for wsb, dstT in ((wq_sb, qT), (wk_sb, kT), (wv_sb, vT)):
                for p in (0, 1):
                    ps = s1psum.tile([128, 512], f32, tag="proj")
                    for ki in range(8):
                        nc.tensor.matmul(
                            ps,
                            lhsT=wsb[:, ki, p * 128 : (p + 1) * 128],
                            rhs=xt[:, ki, :],
                            start=(ki == 0),
                            stop=(ki == 7),
                        )
                    dst = dstT[p][:, t5 * 512 : (t5 + 1) * 512]
                    if ev % 2 == 0:
                        nc.scalar.copy(out=dst, in_=ps)
                    else:
                        nc.vector.tensor_copy(out=dst, in_=ps)
                    ev += 1

    # ------------------------------------------------ stage 2: attention -----
    with tc.tile_pool(name="qkps", bufs=2, space="PSUM") as qkps, \
         tc.tile_pool(name="vtps", bufs=2, space="PSUM") as vtps, \
         tc.tile_pool(name="accps", bufs=1, space="PSUM") as accps, \
         tc.tile_pool(name="zzps", bufs=1, space="PSUM") as zzps, \
         tc.tile_pool(name="epool", bufs=2) as epool, \
         tc.tile_pool(name="rzp", bufs=2) as rzp:

        def stage2():
            # static dilation-group offset of slot sl for each scale
            def G(si, sl):
                return [0, sl // 2, sl, 2 * sl + mcase // 2, 4 * sl + mcase][si]

            for p in (0, 1):
                seg_state = {}

                def segment_pair(si, j):
                    # emit both parities' QK/transpose matmuls interleaved so
                    # the PE row-groups (0-63 vs 64-127) run them concurrently
                    st = {}
                    for par in (0, 1):
                        sl = 2 * p + par
                        hp = par * 64
                        r, w = DILS[si], WS[si]
                        base = j * w + G(si, sl)
                        end = base + 255 * r + 1
                        st[par] = dict(
                            hp=hp,
                            qs=qT[p][hp : hp + 64, base : end : r],
                            ks=kT[p][hp : hp + 64, base : end : r],
                            vs=vT[p][hp : hp + 64, base : end : r],
                            qk=qkps.tile([128, 512], f32, tag="qk", name="qk"),
                        )
                    qms = {0: [], 1: []}
                    for kt in (0, 1):
                        for par in (0, 1):
                            s_ = st[par]
                            if kt == 0:
                                qms[par].append(nc.tensor.matmul(
                                    s_["qk"][:, 0:256], lhsT=s_["ks"][:, 0:128],
                                    rhs=s_["qs"], start=True, stop=False))
                            else:
                                qms[par].append(nc.tensor.matmul(
                                    s_["qk"][:, 256:384],
                                    lhsT=s_["ks"][:, 128:256],
                                    rhs=s_["qs"][:, 128:256],
                                    start=False, stop=True))
                    chain(qms[0])
                    chain(qms[1])
                    for par in (0, 1):
                        segment_tail(si, par, st[par])

                def segment_tail(si, par, s_):
                    hp = s_["hp"]
                    qk = s_["qk"]
                    vs = s_["vs"]
                    # scale 0's e0 is padded to 512 cols of which 256:512 are
                    # zeros: its N=512 start=True AV/Z matmuls then clear the
                    # full 2KB PSUM bank row for the later strided accumulates.
                    e0w = 512 if si == 0 else 256
                    e0 = epool.tile([128, e0w], bf16, tag=f"e0_{si}_{par}",
                                    bufs=ebufs, name="e0")
                    e1 = epool.tile([128, 128], bf16, tag=f"e1_{si}_{par}",
                                    bufs=ebufs, name="e1")
                    if si == 0:
                        nc.vector.memset(e0[:, 256:512], 0.0)
                    nc.scalar.activation(out=e0[:, 0:256], in_=qk[:, 0:256],
                                         func=Exp, scale=0.125)
                    nc.scalar.activation(out=e1, in_=qk[:, 256:384], func=Exp,
                                         scale=0.125)
                    # causal mask on diagonal 128x128 sub-blocks: keep q >= k
                    for msk in (e0[:, 0:128], e1):
                        nc.vector.tensor_mul(msk, msk, trimask)
                    vt = vtps.tile([128, 1024], bf16, tag="vt", name="vt")
                    idsl = ident[hp : hp + 64, hp : hp + 64]
                    vm0 = nc.tensor.matmul(vt[:, 0:64], lhsT=vs[:, 0:128],
                                           rhs=idsl, is_transpose=True,
                                           start=True, stop=False)
                    vm1 = nc.tensor.matmul(vt[:, 64:128], lhsT=vs[:, 128:256],
                                           rhs=idsl, is_transpose=True,
                                           start=False, stop=True)
                    chain([vm0, vm1])
                    vseg = epool.tile([128, 128], bf16, tag=f"vs_{si}_{par}",
                                      bufs=ebufs, name="vseg")
                    nc.scalar.copy(out=vseg, in_=vt[:, 0:128])
                    seg_state[(si, par)] = (e0, e1, vseg)

                for b in range(NBLK):
                    for si in scales_on:
                        if b % DILS[si] == 0:
                            segment_pair(si, b // DILS[si])
                    accs = [accps.tile([128, 512], f32, tag=f"acc{q_}", name="acc")
                            for q_ in (0, 1)]
                    zzs = [zzps.tile([128, 512], f32, tag=f"zz{q_}", name="zz")
                           for q_ in (0, 1)]
                    acc_mms, zz_mms = [], []
                    for si in scales_on:
                        r = DILS[si]
                        L = 256 // r
                        m = b % r
                        for par in (0, 1):
                            sl = 2 * p + par
                            hp = par * 64
                            e0, e1, vseg = seg_state[(si, par)]
                            acc, zz = accs[par], zzs[par]
                            g = G(si, sl)
                            a_ap = acc[hp : hp + 64, g : 256 : r]
                            z_ap = zz[hp : hp + 64, g : 256 : r]
                            if si == 0:
                                a_ap1 = acc[hp : hp + 64, 128:256]
                                z_ap1 = zz[hp : hp + 64, 128:256]
                            else:
                                a_ap1, z_ap1 = a_ap, z_ap
                            first = (si == scales_on[0])
                            last = (si == scales_on[-1])
                            has_kt1 = (si == 0) or (m >= r // 2)
                            if si == 0:
                                r0 = e0[:, 0:512]
                                a_ap = acc[hp : hp + 64, 0:512]
                                z_ap = zz[hp : hp + 64, 0:512]
                            else:
                                r0 = e0[:, m * L : (m + 1) * L]
                            acc_mms.append(nc.tensor.matmul(
                                a_ap, lhsT=vseg[:, 0:64], rhs=r0,
                                start=first, stop=(last and not has_kt1),
                                tile_position=(0, hp)))
                            zz_mms.append(nc.tensor.matmul(
                                z_ap, lhsT=ones64, rhs=r0,
                                start=first, stop=(last and not has_kt1),
                                tile_position=(0, hp)))
                            if has_kt1:
                                if si == 0:
                                    r1 = e1[:, 0:128]
                                else:
                                    r1 = e1[:, m * L - 128 : (m + 1) * L - 128]
                                acc_mms.append(nc.tensor.matmul(
                                    a_ap1, lhsT=vseg[:, 64:128], rhs=r1,
                                    start=False, stop=last, tile_position=(0, hp)))
                                zz_mms.append(nc.tensor.matmul(
                                    z_ap1, lhsT=ones64, rhs=r1,
                                    start=False, stop=last, tile_position=(0, hp)))
                    chain(acc_mms)
                    chain(zz_mms)
                    if 'norm' in parts:
                        # custom-DVE ops only work at partition base 0 on HW:
                        # stage both parities' Z into one tile, single recip.
                        zc = rzp.tile([128, 256], f32, tag="zc", bufs=2,
                                      name="zc")
                        rz = rzp.tile([128, 256], f32, tag="rz", bufs=2,
                                      name="rz")
                        for par in (0, 1):
                            hp = par * 64
                            nc.scalar.copy(out=zc[hp : hp + 64, :],
                                           in_=zzs[par][hp : hp + 64, 0:256])
                        nc.vector.reciprocal_approx_fast(out=rz, in_=zc)
                        for par in (0, 1):
                            hp = par * 64
                            nc.vector.tensor_mul(
                                yT[p][hp : hp + 64, b * 256 : (b + 1) * 256],
                                accs[par][hp : hp + 64, 0:256],
                                rz[hp : hp + 64, :],
                            )

        if 's2' in parts:
            stage2()
        else:
            for p_ in (0, 1):
                nc.vector.memset(yT[p_], 0.5)

    # ------------------------------------------------ stage 3: out proj ------
    with tc.tile_pool(name="s3psum", bufs=2, space="PSUM") as s3psum, \
         tc.tile_pool(name="ostg", bufs=3) as ostg:
        for t in (range(32) if 's3' in parts else []):
            og = ostg.tile([128, 1024], f32, tag="og", name="og", bufs=obufs)
            for nh in (0, 1):
                ps = s3psum.tile([128, 512], f32, tag=f"o{nh}", name="ops")
                for p2 in (0, 1):
                    nc.tensor.matmul(
                        ps,
                        lhsT=yT[p2][:, t * 128 : (t + 1) * 128],
                        rhs=wo_sb[:, p2, nh * 512 : (nh + 1) * 512],
                        start=(p2 == 0), stop=(p2 == 1))
                dst = og[:, nh * 512 : (nh + 1) * 512]
                if nh == 0:
                    nc.scalar.copy(out=dst, in_=ps)
                else:
                    nc.vector.tensor_copy(out=dst, in_=ps)
            nc.sync.dma_start(out=out_d[t * 128 : (t + 1) * 128, :], in_=og)

    if debug:
        bufs = {"qT": qT, "kT": kT, "vT": vT, "yT": yT}
        for nm, tl in bufs.items():
            dd = nc.dram_tensor(f"dbg_{nm}", [2, 128, T], mybir.dt.bfloat16,
                                kind="ExternalOutput")
            for p in (0, 1):
                nc.sync.dma_start(out=dd[p], in_=tl[p][:])


def build_program(mcase, debug=False, scales_on=(0, 1, 2, 3, 4), parts=('s1','s2','norm','s3'), ebufs=4, xbufs=3, obufs=4):
    from contextlib import ExitStack
    import concourse.tile as tile
    from concourse import bacc

    nc = bacc.Bacc("TRN2", target_bir_lowering=False, debug=False, num_devices=2)
    with tile.TileContext(nc) as tc:
        with ExitStack() as ctx:
            _emit(nc, tc, ctx, mcase, debug=debug, scales_on=scales_on, parts=parts, ebufs=ebufs, xbufs=xbufs, obufs=obufs)
    nc.compile()
    return nc


# ---------------------------------------------------------------- host side --
def make_in_maps(inputs):
    bf = ml_dtypes.bfloat16
    x, wq, wk, wv = inputs["x"], inputs["wq"], inputs["wk"], inputs["wv"]
    wo = inputs["wo"]
    ident = np.eye(128, dtype=np.float32).astype(bf)
    # upper-tri in [k, q] orientation: keep q >= k
    trimask = np.triu(np.ones((128, 128), np.float32)).astype(bf)
    in_maps = []
    for c in range(NCORES):
        b, m = c // 4, c % 4
        heads = [4 * hl + m for hl in range(4)]
        in_maps.append({
            "xT": np.ascontiguousarray(np.asarray(x)[b].T).astype(bf),
            "wq": np.ascontiguousarray(
                np.asarray(wq).reshape(E, H, D)[:, heads].reshape(E, 256)).astype(bf),
            "wk": np.ascontiguousarray(
                np.asarray(wk).reshape(E, H, D)[:, heads].reshape(E, 256)).astype(bf),
            "wv": np.ascontiguousarray(
                np.asarray(wv).reshape(E, H, D)[:, heads].reshape(E, 256)).astype(bf),
            "wo": np.ascontiguousarray(
                np.asarray(wo).reshape(H, D, E)[heads].reshape(256, E)).astype(bf),
            "ident": ident,
            "trimask": trimask,
        })
    return in_maps


class GroupRuntime:
    """Cached-jit runner for one head-group program on devices [m, m+4]."""

    def __init__(self, nc, devices):
        import jax
        import concourse.mybir as mybir
        from concourse import bass2jax
        from jax.experimental.shard_map import shard_map
        from jax.sharding import Mesh, PartitionSpec

        bass2jax.install_neuronx_cc_hook()
        self.jax = jax
        self.nc = nc
        in_names, out_names, out_avals, zero_outs = [], [], [], []
        pid_name = nc.partition_id_tensor.name if nc.partition_id_tensor else None
        for alloc in nc.m.functions[0].allocations:
            if not isinstance(alloc, mybir.MemoryLocationSet):
                continue
            name = alloc.memorylocations[0].name
            if alloc.kind == "ExternalInput":
                if name != pid_name:
                    in_names.append(name)
            elif alloc.kind == "ExternalOutput":
                shape = tuple(alloc.tensor_shape)
                dtype = mybir.dt.np(alloc.dtype)
                out_names.append(name)
                out_avals.append(jax.core.ShapedArray(shape, dtype))
                zero_outs.append(np.zeros(shape, dtype))
        self.in_names, self.out_names = in_names, out_names
        n_params, n_outs = len(in_names), len(out_names)
        self.n_params, self.n_outs = n_params, n_outs
        self.zero_outs = zero_outs
        body_names = in_names + out_names + ([pid_name] if pid_name else [])

        def _body(*args):
            operands = list(args)
            if pid_name is not None:
                operands.append(bass2jax.partition_id_tensor())
            outs = bass2jax._bass_exec_p.bind(
                *operands,
                out_avals=tuple(out_avals),
                in_names=tuple(body_names),
                out_names=tuple(out_names),
                lowering_input_output_aliases=(),
                sim_require_finite=False,
                sim_require_nnan=False,
                nc=nc,
            )
            return tuple(outs)

        self.n_dev = len(devices)
        self.mesh = Mesh(np.asarray(devices), ("core",))
        in_specs = (PartitionSpec("core"),) * (n_params + n_outs)
        out_specs = (PartitionSpec("core"),) * n_outs
        donate = tuple(range(n_params, n_params + n_outs))
        self.fn = jax.jit(
            shard_map(_body, mesh=self.mesh, in_specs=in_specs,
                      out_specs=out_specs, check_rep=False),
            donate_argnums=donate, keep_unused=True)

    def prep(self, group_maps):
        from jax.sharding import NamedSharding, PartitionSpec
        sh = NamedSharding(self.mesh, PartitionSpec("core"))
        np_in = [
            self.jax.device_put(
                np.concatenate([np.asarray(gm[n]) for gm in group_maps], axis=0), sh)
            for n in self.in_names
        ]
        zeros = [
            self.jax.device_put(
                np.zeros((self.n_dev * z.shape[0], *z.shape[1:]), z.dtype), sh)
            for z in self.zero_outs
        ]
        return np_in, zeros


class Runtime:
    """Four per-head-group programs dispatched concurrently on 8 cores."""

    def __init__(self, debug=False):
        import jax
        self.jax = jax
        devs = jax.devices()
        assert len(devs) >= NCORES
        self.groups = []
        for m in range(4):
            nc = build_program(m, debug=debug)
            self.groups.append(GroupRuntime(nc, [devs[m], devs[m + 4]]))

    def run(self, in_maps):
        outs = []
        for m, grt in enumerate(self.groups):
            np_in, zeros = grt.prep([in_maps[m], in_maps[m + 4]])
            outs.append(grt.fn(*np_in, *zeros))
        self.jax.block_until_ready(outs)
        res = [dict() for _ in range(NCORES)]
        for m, grt in enumerate(self.groups):
            for i, n in enumerate(grt.out_names):
                a = np.asarray(outs[m][i]).reshape(2, *grt.zero_outs[i].shape)
                res[m][n] = a[0]
                res[m + 4][n] = a[1]
        return res

    def time(self, in_maps, iters=10):
        """Min wall time of back-to-back dispatches with device-resident data."""
        import time as _t
        prepped = []
        for m, grt in enumerate(self.groups):
            np_in, zeros = grt.prep([in_maps[m], in_maps[m + 4]])
            prepped.append((grt, np_in, list(grt.fn(*np_in, *zeros))))
        self.jax.block_until_ready([p[2] for p in prepped])
        times = []
        for _ in range(iters):
            t0 = _t.perf_counter()
            nxt = []
            for grt, np_in, prev in prepped:
                nxt.append(list(grt.fn(*np_in, *prev)))
            self.jax.block_until_ready(nxt)
            times.append(_t.perf_counter() - t0)
            prepped = [(g, ni, nx) for (g, ni, _), nx in zip(prepped, nxt)]
        return min(times)


def _get_runtime():
    global _RUNTIME
    if _RUNTIME is None:
        _RUNTIME = Runtime()
    return _RUNTIME


def _numpy_fallback(inputs):
    x = np.asarray(inputs["x"], np.float32)
    wq, bq = np.asarray(inputs["wq"]), np.asarray(inputs["bq"])
    wk, bk = np.asarray(inputs["wk"]), np.asarray(inputs["bk"])
    wv, bv = np.asarray(inputs["wv"]), np.asarray(inputs["bv"])
    wo, bo = np.asarray(inputs["wo"]), np.asarray(inputs["bo"])
    q = (x @ wq + bq).reshape(B, T, H, D) * (D ** -0.5)
    k = (x @ wk + bk).reshape(B, T, H, D)
    v = (x @ wv + bv).reshape(B, T, H, D)
    y = np.zeros((B, T, H, D), np.float32)
    zz = np.zeros((B, T, H), np.float32)
    for w, r in zip(WS, DILS):
        s = w // r
        tri = np.tril(np.ones((s, s), np.float32))
        for h in range(H):
            g = h // (H // r)
            for j in range(T // w):
                pos = j * w + g + r * np.arange(s)
                for b in range(B):
                    sc = q[b, pos, h] @ k[b, pos, h].T
                    e = np.exp(sc) * tri
                    y[b, pos, h] += e @ v[b, pos, h]
                    zz[b, pos, h] += e.sum(1)
    y = y / zz[..., None]
    return y.reshape(B, T, E) @ wo + bo


def kernel(**inputs):
    if any(np.abs(np.asarray(inputs[b])).max() > 0 for b in ("bq", "bk", "bv")):
        return _numpy_fallback(inputs)
    rt = _get_runtime()
    res = rt.run(make_in_maps(inputs))
    out = np.zeros((B, T, E), np.float32)
    for c in range(NCORES):
        out[c // 4] += res[c]["out"]
    out += np.asarray(inputs["bo"], np.float32)
    return out


# revision 34
# speedup vs baseline: 1.0007x; 1.0007x over previous
"""Trainium2 Bass kernel for LongNet-style dilated attention (B=2, T=4096, E=1024, H=16).

Sharding: 8 cores = 2 batches x 4 head-groups. Core (b, m) handles batch b and
heads {m, 4+m, 8+m, 12+m}. One SPMD program: the attention stage is emitted 4x
under tc.If(m == c) branches so every dilation-group offset is compile-time
static (register-offset APs only work at partition base 0 on TRN2, so dynamic
slicing is not usable for the odd head slots that live at partitions 64-127).

Math: for every scale, segment sparse length s = w/r = 256, so each (head,
scale, segment) is one [256 x 256 x 64] causal attention block. The reference's
lse-softmax merge over scales reduces algebraically to
    y[t,h] = (sum_i rawout_i[t,h]) / (sum_i Z_i[t,h])
over covering scales, with rawout = exp(s)@v and Z = rowsum(exp(s)),
accumulated directly in PSUM by aligning all scales' segment grids on 256-wide
t-blocks.  No exp/log at merge time; no per-scale normalization; Z comes from a
col-tiled all-ones matmul replicated across 64 partitions so the final
normalize is a partition-aligned reciprocal+multiply.
"""

import numpy as np
import ml_dtypes

B, T, E, H, D = 2, 4096, 1024, 16, 64
WS = (256, 512, 1024, 2048, 4096)
DILS = (1, 2, 4, 8, 16)
NBLK = 16
NCORES = 8

_RUNTIME = None


# ---------------------------------------------------------------- program ----
def _emit(nc, tc, ctx, mcase, debug=False, scales_on=(0,1,2,3,4), parts=('s1','s2','norm','s3'), ebufs=4, xbufs=3, obufs=4):
    import concourse.mybir as mybir
    from concourse.bass import _add_dep_helper

    def chain(insts):
        # enforce PE program order for matmuls sharing a PSUM accumulation bank
        for a, b_ in zip(insts[1:], insts[:-1]):
            _add_dep_helper(a.ins, b_.ins, False, "psum group order")

    bf16 = mybir.dt.bfloat16
    f32 = mybir.dt.float32
    Exp = mybir.ActivationFunctionType.Exp

    xT = nc.dram_tensor("xT", [E, T], bf16, kind="ExternalInput")
    wq_d = nc.dram_tensor("wq", [E, 256], bf16, kind="ExternalInput")
    wk_d = nc.dram_tensor("wk", [E, 256], bf16, kind="ExternalInput")
    wv_d = nc.dram_tensor("wv", [E, 256], bf16, kind="ExternalInput")
    wo_d = nc.dram_tensor("wo", [256, E], bf16, kind="ExternalInput")
    id_d = nc.dram_tensor("ident", [128, 128], bf16, kind="ExternalInput")
    tri_d = nc.dram_tensor("trimask", [128, 128], bf16, kind="ExternalInput")
    out_d = nc.dram_tensor("out", [T, E], f32, kind="ExternalOutput")

    consts = ctx.enter_context(tc.tile_pool(name="consts", bufs=1))
    wq_sb = consts.tile([128, 8, 256], bf16)
    wk_sb = consts.tile([128, 8, 256], bf16)
    wv_sb = consts.tile([128, 8, 256], bf16)
    wo_sb = consts.tile([128, 2, 1024], bf16)
    ident = consts.tile([128, 128], bf16)
    trimask = consts.tile([128, 128], bf16)
    ones64 = consts.tile([128, 64], bf16)
    nc.sync.dma_start(out=wq_sb, in_=wq_d.rearrange("(k p) m -> p k m", p=128))
    nc.sync.dma_start(out=wk_sb, in_=wk_d.rearrange("(k p) m -> p k m", p=128))
    nc.sync.dma_start(out=wv_sb, in_=wv_d.rearrange("(k p) m -> p k m", p=128))
    nc.sync.dma_start(out=wo_sb, in_=wo_d.rearrange("(k p) m -> p k m", p=128))
    nc.sync.dma_start(out=ident, in_=id_d[:])
    nc.sync.dma_start(out=trimask, in_=tri_d[:])
    nc.vector.memset(ones64, 1.0)

    big = ctx.enter_context(tc.tile_pool(name="big", bufs=1))
    qT = [big.tile([128, T], bf16, tag=f"qT{p}", name=f"qT{p}") for p in (0, 1)]
    kT = [big.tile([128, T], bf16, tag=f"kT{p}", name=f"kT{p}") for p in (0, 1)]
    vT = [big.tile([128, T], bf16, tag=f"vT{p}", name=f"vT{p}") for p in (0, 1)]
    yT = [big.tile([128, T], bf16, tag=f"yT{p}", name=f"yT{p}") for p in (0, 1)]

    # ------------------------------------------------ stage 1: projections ---
    if 's1' not in parts:
        for tl in (*qT, *kT, *vT):
            nc.sync.dma_start(out=tl, in_=xT[0:128, :])
    with tc.tile_pool(name="s1psum", bufs=2, space="PSUM") as s1psum, \
         tc.tile_pool(name="xin", bufs=2) as xin:
        xTr = xT.rearrange("(k p) t -> p k t", p=128)
        for t5 in (range(8) if 's1' in parts else []):
            xt = xin.tile([128, 8, 512], bf16, tag="xt", bufs=xbufs)
            nc.sync.dma_start(out=xt, in_=xTr[:, :, t5 * 512 : (t5 + 1) * 512])
            ev = 0
            for wsb, dstT in ((wq_sb, qT), (wk_sb, kT), (wv_sb, vT)):
                for p in (0, 1):
                    ps = s1psum.tile([128, 512], f32, tag="proj")
                    for ki in range(8):
                        nc.tensor.matmul(
                            ps,
                            lhsT=wsb[:, ki, p * 128 : (p + 1) * 128],
                            rhs=xt[:, ki, :],
                            start=(ki == 0),
                            stop=(ki == 7),
                        )
                    dst = dstT[p][:, t5 * 512 : (t5 + 1) * 512]
                    if ev % 2 == 0:
                        nc.scalar.copy(out=dst, in_=ps)
                    else:
                        nc.vector.tensor_copy(out=dst, in_=ps)
                    ev += 1

    # ------------------------------------------------ stage 2: attention -----
    with tc.tile_pool(name="qkps", bufs=2, space="PSUM") as qkps, \
         tc.tile_pool(name="vtps", bufs=2, space="PSUM") as vtps, \
         tc.tile_pool(name="accps", bufs=1, space="PSUM") as accps, \
         tc.tile_pool(name="zzps", bufs=1, space="PSUM") as zzps, \
         tc.tile_pool(name="epool", bufs=2) as epool, \
         tc.tile_pool(name="rzp", bufs=2) as rzp:

        def stage2():
            # static dilation-group offset of slot sl for each scale
            def G(si, sl):
                return [0, sl // 2, sl, 2 * sl + mcase // 2, 4 * sl + mcase][si]

            for p in (0, 1):
                seg_state = {}

                def segment_pair(si, j):
                    # emit both parities' QK/transpose matmuls interleaved so
                    # the PE row-groups (0-63 vs 64-127) run them concurrently
                    st = {}
                    for par in (0, 1):
                        sl = 2 * p + par
                        hp = par * 64
                        r, w = DILS[si], WS[si]
                        base = j * w + G(si, sl)
                        end = base + 255 * r + 1
                        st[par] = dict(
                            hp=hp,
                            qs=qT[p][hp : hp + 64, base : end : r],
                            ks=kT[p][hp : hp + 64, base : end : r],
                            vs=vT[p][hp : hp + 64, base : end : r],
                            qk=qkps.tile([128, 512], f32, tag="qk", name="qk"),
                        )
                    qms = {0: [], 1: []}
                    for kt in (0, 1):
                        for par in (0, 1):
                            s_ = st[par]
                            if kt == 0:
                                qms[par].append(nc.tensor.matmul(
                                    s_["qk"][:, 0:256], lhsT=s_["ks"][:, 0:128],
                                    rhs=s_["qs"], start=True, stop=False))
                            else:
                                qms[par].append(nc.tensor.matmul(
                                    s_["qk"][:, 256:384],
                                    lhsT=s_["ks"][:, 128:256],
                                    rhs=s_["qs"][:, 128:256],
                                    start=False, stop=True))
                    chain(qms[0])
                    chain(qms[1])
                    vms = {0: [], 1: []}
                    for par in (0, 1):
                        st[par]["vt"] = vtps.tile([128, 1024], bf16, tag="vt",
                                                  name="vt")
                    for kt in (0, 1):
                        for par in (0, 1):
                            s_ = st[par]
                            hp = s_["hp"]
                            idsl = ident[hp : hp + 64, hp : hp + 64]
                            if kt == 0:
                                vms[par].append(nc.tensor.matmul(
                                    s_["vt"][:, 0:64], lhsT=s_["vs"][:, 0:128],
                                    rhs=idsl, is_transpose=True,
                                    start=True, stop=False))
                            else:
                                vms[par].append(nc.tensor.matmul(
                                    s_["vt"][:, 64:128],
                                    lhsT=s_["vs"][:, 128:256],
                                    rhs=idsl, is_transpose=True,
                                    start=False, stop=True))
                    chain(vms[0])
                    chain(vms[1])
                    for par in (0, 1):
                        segment_tail(si, par, st[par])

                def segment_tail(si, par, s_):
                    hp = s_["hp"]
                    qk = s_["qk"]
                    vs = s_["vs"]
                    # scale 0's e0 is padded to 512 cols of which 256:512 are
                    # zeros: its N=512 start=True AV/Z matmuls then clear the
                    # full 2KB PSUM bank row for the later strided accumulates.
                    e0w = 512 if si == 0 else 256
                    e0 = epool.tile([128, e0w], bf16, tag=f"e0_{si}_{par}",
                                    bufs=ebufs, name="e0")
                    e1 = epool.tile([128, 128], bf16, tag=f"e1_{si}_{par}",
                                    bufs=ebufs, name="e1")
                    if si == 0:
                        nc.vector.memset(e0[:, 256:512], 0.0)
                    nc.scalar.activation(out=e0[:, 0:256], in_=qk[:, 0:256],
                                         func=Exp, scale=0.125)
                    nc.scalar.activation(out=e1, in_=qk[:, 256:384], func=Exp,
                                         scale=0.125)
                    # causal mask on diagonal 128x128 sub-blocks: keep q >= k
                    for msk in (e0[:, 0:128], e1):
                        nc.vector.tensor_mul(msk, msk, trimask)
                    vt = s_["vt"]
                    vseg = epool.tile([128, 128], bf16, tag=f"vs_{si}_{par}",
                                      bufs=ebufs, name="vseg")
                    nc.scalar.copy(out=vseg, in_=vt[:, 0:128])
                    seg_state[(si, par)] = (e0, e1, vseg)

                for b in range(NBLK):
                    for si in scales_on:
                        if b % DILS[si] == 0:
                            segment_pair(si, b // DILS[si])
                    accs = [accps.tile([128, 512], f32, tag=f"acc{q_}", name="acc")
                            for q_ in (0, 1)]
                    zzs = [zzps.tile([128, 512], f32, tag=f"zz{q_}", name="zz")
                           for q_ in (0, 1)]
                    acc_mms, zz_mms = [], []
                    for si in scales_on:
                        r = DILS[si]
                        L = 256 // r
                        m = b % r
                        for par in (0, 1):
                            sl = 2 * p + par
                            hp = par * 64
                            e0, e1, vseg = seg_state[(si, par)]
                            acc, zz = accs[par], zzs[par]
                            g = G(si, sl)
                            a_ap = acc[hp : hp + 64, g : 256 : r]
                            z_ap = zz[hp : hp + 64, g : 256 : r]
                            if si == 0:
                                a_ap1 = acc[hp : hp + 64, 128:256]
                                z_ap1 = zz[hp : hp + 64, 128:256]
                            else:
                                a_ap1, z_ap1 = a_ap, z_ap
                            first = (si == scales_on[0])
                            last = (si == scales_on[-1])
                            has_kt1 = (si == 0) or (m >= r // 2)
                            if si == 0:
                                r0 = e0[:, 0:512]
                                a_ap = acc[hp : hp + 64, 0:512]
                                z_ap = zz[hp : hp + 64, 0:512]
                            else:
                                r0 = e0[:, m * L : (m + 1) * L]
                            acc_mms.append(nc.tensor.matmul(
                                a_ap, lhsT=vseg[:, 0:64], rhs=r0,
                                start=first, stop=(last and not has_kt1),
                                tile_position=(0, hp)))
                            zz_mms.append(nc.tensor.matmul(
                                z_ap, lhsT=ones64, rhs=r0,
                                start=first, stop=(last and not has_kt1),
                                tile_position=(0, hp)))
                            if has_kt1:
                                if si == 0:
                                    r1 = e1[:, 0:128]
                                else:
                                    r1 = e1[:, m * L - 128 : (m + 1) * L - 128]
                                acc_mms.append(nc.tensor.matmul(
                                    a_ap1, lhsT=vseg[:, 64:128], rhs=r1,
                                    start=False, stop=last, tile_position=(0, hp)))
                                zz_mms.append(nc.tensor.matmul(
                                    z_ap1, lhsT=ones64, rhs=r1,
                                    start=False, stop=last, tile_position=(0, hp)))
                    chain(acc_mms)
                    chain(zz_mms)
                    if 'norm' in parts:
                        # custom-DVE ops only work at partition base 0 on HW:
                        # stage both parities' Z into one tile, single recip.
                        zc = rzp.tile([128, 256], f32, tag="zc", bufs=2,
                                      name="zc")
                        rz = rzp.tile([128, 256], f32, tag="rz", bufs=2,
                                      name="rz")
                        for par in (0, 1):
                            hp = par * 64
                            nc.scalar.copy(out=zc[hp : hp + 64, :],
                                           in_=zzs[par][hp : hp + 64, 0:256])
                        nc.vector.reciprocal_approx_fast(out=rz, in_=zc)
                        for par in (0, 1):
                            hp = par * 64
                            nc.vector.tensor_mul(
                                yT[p][hp : hp + 64, b * 256 : (b + 1) * 256],
                                accs[par][hp : hp + 64, 0:256],
                                rz[hp : hp + 64, :],
                            )

        if 's2' in parts:
            stage2()
        else:
            for p_ in (0, 1):
                nc.vector.memset(yT[p_], 0.5)

    # ------------------------------------------------ stage 3: out proj ------
    with tc.tile_pool(name="s3psum", bufs=2, space="PSUM") as s3psum, \
         tc.tile_pool(name="ostg", bufs=3) as ostg:
        for t in (range(32) if 's3' in parts else []):
            og = ostg.tile([128, 1024], f32, tag="og", name="og", bufs=obufs)
            for nh in (0, 1):
                ps = s3psum.tile([128, 512], f32, tag=f"o{nh}", name="ops")
                for p2 in (0, 1):
                    nc.tensor.matmul(
                        ps,
                        lhsT=yT[p2][:, t * 128 : (t + 1) * 128],
                        rhs=wo_sb[:, p2, nh * 512 : (nh + 1) * 512],
                        start=(p2 == 0), stop=(p2 == 1))
                dst = og[:, nh * 512 : (nh + 1) * 512]
                if nh == 0:
                    nc.scalar.copy(out=dst, in_=ps)
                else:
                    nc.vector.tensor_copy(out=dst, in_=ps)
            nc.sync.dma_start(out=out_d[t * 128 : (t + 1) * 128, :], in_=og)

    if debug:
        bufs = {"qT": qT, "kT": kT, "vT": vT, "yT": yT}
        for nm, tl in bufs.items():
            dd = nc.dram_tensor(f"dbg_{nm}", [2, 128, T], mybir.dt.bfloat16,
                                kind="ExternalOutput")
            for p in (0, 1):
                nc.sync.dma_start(out=dd[p], in_=tl[p][:])


def build_program(mcase, debug=False, scales_on=(0, 1, 2, 3, 4), parts=('s1','s2','norm','s3'), ebufs=4, xbufs=3, obufs=4):
    from contextlib import ExitStack
    import concourse.tile as tile
    from concourse import bacc

    nc = bacc.Bacc("TRN2", target_bir_lowering=False, debug=False, num_devices=2)
    with tile.TileContext(nc) as tc:
        with ExitStack() as ctx:
            _emit(nc, tc, ctx, mcase, debug=debug, scales_on=scales_on, parts=parts, ebufs=ebufs, xbufs=xbufs, obufs=obufs)
    nc.compile()
    return nc


# ---------------------------------------------------------------- host side --
def make_in_maps(inputs):
    bf = ml_dtypes.bfloat16
    x, wq, wk, wv = inputs["x"], inputs["wq"], inputs["wk"], inputs["wv"]
    wo = inputs["wo"]
    ident = np.eye(128, dtype=np.float32).astype(bf)
    # upper-tri in [k, q] orientation: keep q >= k
    trimask = np.triu(np.ones((128, 128), np.float32)).astype(bf)
    in_maps = []
    for c in range(NCORES):
        b, m = c // 4, c % 4
        heads = [4 * hl + m for hl in range(4)]
        in_maps.append({
            "xT": np.ascontiguousarray(np.asarray(x)[b].T).astype(bf),
            "wq": np.ascontiguousarray(
                np.asarray(wq).reshape(E, H, D)[:, heads].reshape(E, 256)).astype(bf),
            "wk": np.ascontiguousarray(
                np.asarray(wk).reshape(E, H, D)[:, heads].reshape(E, 256)).astype(bf),
            "wv": np.ascontiguousarray(
                np.asarray(wv).reshape(E, H, D)[:, heads].reshape(E, 256)).astype(bf),
            "wo": np.ascontiguousarray(
                np.asarray(wo).reshape(H, D, E)[heads].reshape(256, E)).astype(bf),
            "ident": ident,
            "trimask": trimask,
        })
    return in_maps


class GroupRuntime:
    """Cached-jit runner for one head-group program on devices [m, m+4]."""

    def __init__(self, nc, devices):
        import jax
        import concourse.mybir as mybir
        from concourse import bass2jax
        from jax.experimental.shard_map import shard_map
        from jax.sharding import Mesh, PartitionSpec

        bass2jax.install_neuronx_cc_hook()
        self.jax = jax
        self.nc = nc
        in_names, out_names, out_avals, zero_outs = [], [], [], []
        pid_name = nc.partition_id_tensor.name if nc.partition_id_tensor else None
        for alloc in nc.m.functions[0].allocations:
            if not isinstance(alloc, mybir.MemoryLocationSet):
                continue
            name = alloc.memorylocations[0].name
            if alloc.kind == "ExternalInput":
                if name != pid_name:
                    in_names.append(name)
            elif alloc.kind == "ExternalOutput":
                shape = tuple(alloc.tensor_shape)
                dtype = mybir.dt.np(alloc.dtype)
                out_names.append(name)
                out_avals.append(jax.core.ShapedArray(shape, dtype))
                zero_outs.append(np.zeros(shape, dtype))
        self.in_names, self.out_names = in_names, out_names
        n_params, n_outs = len(in_names), len(out_names)
        self.n_params, self.n_outs = n_params, n_outs
        self.zero_outs = zero_outs
        body_names = in_names + out_names + ([pid_name] if pid_name else [])

        def _body(*args):
            operands = list(args)
            if pid_name is not None:
                operands.append(bass2jax.partition_id_tensor())
            outs = bass2jax._bass_exec_p.bind(
                *operands,
                out_avals=tuple(out_avals),
                in_names=tuple(body_names),
                out_names=tuple(out_names),
                lowering_input_output_aliases=(),
                sim_require_finite=False,
                sim_require_nnan=False,
                nc=nc,
            )
            return tuple(outs)

        self.n_dev = len(devices)
        self.mesh = Mesh(np.asarray(devices), ("core",))
        in_specs = (PartitionSpec("core"),) * (n_params + n_outs)
        out_specs = (PartitionSpec("core"),) * n_outs
        donate = tuple(range(n_params, n_params + n_outs))
        self.fn = jax.jit(
            shard_map(_body, mesh=self.mesh, in_specs=in_specs,
                      out_specs=out_specs, check_rep=False),
            donate_argnums=donate, keep_unused=True)

    def prep(self, group_maps):
        from jax.sharding import NamedSharding, PartitionSpec
        sh = NamedSharding(self.mesh, PartitionSpec("core"))
        np_in = [
            self.jax.device_put(
                np.concatenate([np.asarray(gm[n]) for gm in group_maps], axis=0), sh)
            for n in self.in_names
        ]
        zeros = [
            self.jax.device_put(
                np.zeros((self.n_dev * z.shape[0], *z.shape[1:]), z.dtype), sh)
            for z in self.zero_outs
        ]
        return np_in, zeros


class Runtime:
    """Four per-head-group programs dispatched concurrently on 8 cores."""

    def __init__(self, debug=False):
        import jax
        self.jax = jax
        devs = jax.devices()
        assert len(devs) >= NCORES
        self.groups = []
        for m in range(4):
            nc = build_program(m, debug=debug)
            self.groups.append(GroupRuntime(nc, [devs[m], devs[m + 4]]))

    def run(self, in_maps):
        outs = []
        for m, grt in enumerate(self.groups):
            np_in, zeros = grt.prep([in_maps[m], in_maps[m + 4]])
            outs.append(grt.fn(*np_in, *zeros))
        self.jax.block_until_ready(outs)
        res = [dict() for _ in range(NCORES)]
        for m, grt in enumerate(self.groups):
            for i, n in enumerate(grt.out_names):
                a = np.asarray(outs[m][i]).reshape(2, *grt.zero_outs[i].shape)
                res[m][n] = a[0]
                res[m + 4][n] = a[1]
        return res

    def time(self, in_maps, iters=10):
        """Min wall time of back-to-back dispatches with device-resident data."""
        import time as _t
        prepped = []
        for m, grt in enumerate(self.groups):
            np_in, zeros = grt.prep([in_maps[m], in_maps[m + 4]])
            prepped.append((grt, np_in, list(grt.fn(*np_in, *zeros))))
        self.jax.block_until_ready([p[2] for p in prepped])
        times = []
        for _ in range(iters):
            t0 = _t.perf_counter()
            nxt = []
            for grt, np_in, prev in prepped:
                nxt.append(list(grt.fn(*np_in, *prev)))
            self.jax.block_until_ready(nxt)
            times.append(_t.perf_counter() - t0)
            prepped = [(g, ni, nx) for (g, ni, _), nx in zip(prepped, nxt)]
        return min(times)


def _get_runtime():
    global _RUNTIME
    if _RUNTIME is None:
        _RUNTIME = Runtime()
    return _RUNTIME


def _numpy_fallback(inputs):
    x = np.asarray(inputs["x"], np.float32)
    wq, bq = np.asarray(inputs["wq"]), np.asarray(inputs["bq"])
    wk, bk = np.asarray(inputs["wk"]), np.asarray(inputs["bk"])
    wv, bv = np.asarray(inputs["wv"]), np.asarray(inputs["bv"])
    wo, bo = np.asarray(inputs["wo"]), np.asarray(inputs["bo"])
    q = (x @ wq + bq).reshape(B, T, H, D) * (D ** -0.5)
    k = (x @ wk + bk).reshape(B, T, H, D)
    v = (x @ wv + bv).reshape(B, T, H, D)
    y = np.zeros((B, T, H, D), np.float32)
    zz = np.zeros((B, T, H), np.float32)
    for w, r in zip(WS, DILS):
        s = w // r
        tri = np.tril(np.ones((s, s), np.float32))
        for h in range(H):
            g = h // (H // r)
            for j in range(T // w):
                pos = j * w + g + r * np.arange(s)
                for b in range(B):
                    sc = q[b, pos, h] @ k[b, pos, h].T
                    e = np.exp(sc) * tri
                    y[b, pos, h] += e @ v[b, pos, h]
                    zz[b, pos, h] += e.sum(1)
    y = y / zz[..., None]
    return y.reshape(B, T, E) @ wo + bo


def kernel(**inputs):
    if any(np.abs(np.asarray(inputs[b])).max() > 0 for b in ("bq", "bk", "bv")):
        return _numpy_fallback(inputs)
    rt = _get_runtime()
    res = rt.run(make_in_maps(inputs))
    out = np.zeros((B, T, E), np.float32)
    for c in range(NCORES):
        out[c // 4] += res[c]["out"]
    out += np.asarray(inputs["bo"], np.float32)
    return out
